# revision 8
# baseline (speedup 1.0000x reference)
"""LIF forward recurrence on 8 Trainium2 NeuronCores — v10.

Input  x: (T=16, B=128, N=16384) float32, time-major.
    m[t] = tau * v[t-1] + x[t]      tau = 0.5
    y[t] = (m[t] >= 1)              spike
    v[t] = m[t] * (1 - y[t])        hard reset

Sharding: N split 8 ways (2048 per core); per-neuron recurrence, no
cross-core traffic.  Host re-lays each shard as (B, T, NSH) so chunked
DMAs read long contiguous runs per SBUF partition row.

Engine layout (measured costs per [128x2048] f32 tile):
  DVE    v = (m < 1) * m            stt 2.29us   } the serial chain,
         m' = 0.5*v + x[t+1]        stt 2.29us   } back-to-back on DVE
  Act    a = Sign(1 - m)            2.0us  off-chain
         y = Sign(1 - a) -> uint8   2.0us  exact {0,1}, incl. m == 1
  Sync   y[t] streamed out per step (uint8, 0.25 MB/step)
The DVE pair is the hard floor: fp32 two-tensor ops have no fast DVE
mode, Act can't combine two tensors, GpSimd elementwise ops poison DVE
throughput when co-running (measured ~0.66 vs 0.96 col/ns), and PE can
only accumulate into PSUM (no DMA in/out of PSUM).  Total DMA
(16.8 MB in + 4.2 MB out = ~60us) hides under the 71us DVE stream;
everything else is ramp/tail, minimized by issuing the first input
chunk from the Act queue right after the init barrier and streaming
spikes out every step (no end-of-kernel drain).

Output is bit-exact vs the f32 reference (the double-Sign gives exact
0/1 including m == 1.0, and the v-chain rounds identically).
"""

import numpy as np

import concourse.bass as bass
import concourse.mybir as mybir
from concourse.bass_utils import run_bass_kernel_spmd
from concourse.mybir import AluOpType
from concourse.tile import TileContext

T, B, N = 16, 128, 16384
NCORES = 8
NSH = N // NCORES  # 2048 neurons per core
TAU = 0.5
V_TH = 1.0

IN_CHUNKS = [1, 1, 2, 4, 4, 2, 1, 1]

_cached_nc = None


def _split_multiwaits(nc):
    """Walrus codegen supports only ONE sync-wait per instruction; Tile
    sometimes attaches more.  Move extras onto same-engine NoOps."""
    multi_ok = (mybir.InstEventSemaphore, mybir.InstNoOp)
    for f in nc.m.functions:
        for b in f.blocks:
            new_insts = []
            for inst in b.instructions:
                si = inst.sync_info
                if (
                    not isinstance(inst, multi_ok)
                    and si is not None
                    and len(si.on_wait) > 1
                ):
                    waits = list(si.on_wait)
                    for j, w in enumerate(waits[:-1]):
                        new_insts.append(
                            mybir.InstNoOp(
                                name=f"{inst.name}_presync{j}",
                                engine=inst.engine,
                                sync_info=mybir.SyncInfo(on_wait=[w], on_update=[]),
                            )
                        )
                    inst.sync_info = mybir.SyncInfo(
                        on_wait=[waits[-1]], on_update=list(si.on_update)
                    )
                new_insts.append(inst)
            b.instructions = new_insts


def _build():
    nc = bass.Bass(trn_type="TRN2")
    x = nc.dram_tensor("x", [B, T, NSH], mybir.dt.float32, kind="ExternalInput")
    y = nc.dram_tensor("y", [B, T, NSH], mybir.dt.uint8, kind="ExternalOutput")

    with TileContext(nc) as tc:
        with (
            tc.tile_pool(name="xin", bufs=2) as xin_pool,
            tc.tile_pool(name="mst", bufs=1) as m_pool,
            tc.tile_pool(name="vst", bufs=1) as v_pool,
            tc.tile_pool(name="sst", bufs=1) as s_pool,
        ):
            # chunk tiles; first chunk issued from the Act queue (starts
            # right after the init barrier, ~4us before GpSimd's DGE
            # preamble finishes), the rest from GpSimd's SWDGE path.
            xt_tiles = {}
            xts = []
            t0 = 0
            for ci, w in enumerate(IN_CHUNKS):
                xt = xin_pool.tile(
                    [B, 4, NSH], mybir.dt.float32, tag="xt", name=f"xt{ci}"
                )
                xts.append((xt, t0, w))
                for k in range(w):
                    xt_tiles[t0 + k] = xt[:, k, :]
                t0 += w
            for (xt, a0, w), eng in zip(xts, [nc.scalar, nc.scalar]):
                eng.dma_start(out=xt[:, :w, :], in_=x[:, a0 : a0 + w, :])
            for xt, a0, w in xts[2:]:
                nc.gpsimd.dma_start(out=xt[:, :w, :], in_=x[:, a0 : a0 + w, :])

            m_cur = xt_tiles[0]
            for t in range(T):
                # spikes, exact 0/1 via double Sign (correct at m == 1)
                at = s_pool.tile(
                    [B, NSH], mybir.dt.float32, tag="a", bufs=2, name=f"a_{t}"
                )
                nc.scalar.activation(
                    at[:], m_cur, mybir.ActivationFunctionType.Sign,
                    bias=V_TH, scale=-1.0,
                )
                yt = s_pool.tile(
                    [B, NSH], mybir.dt.uint8, tag="y", bufs=3, name=f"y_{t}"
                )
                nc.scalar.activation(
                    yt[:], at[:], mybir.ActivationFunctionType.Sign,
                    bias=1.0, scale=-1.0,
                )
                nc.sync.dma_start(out=y[:, t, :], in_=yt[:])
                if t == T - 1:
                    continue
                # the serial chain, back-to-back on DVE
                vt = v_pool.tile(
                    [B, NSH], mybir.dt.float32, tag="v", bufs=2, name=f"v_{t}"
                )
                nc.vector.scalar_tensor_tensor(
                    vt[:], m_cur, V_TH, m_cur, AluOpType.is_lt, AluOpType.mult
                )
                mt = m_pool.tile(
                    [B, NSH], mybir.dt.float32, tag="m", bufs=2,
                    name=f"m_{t + 1}",
                )
                nc.vector.scalar_tensor_tensor(
                    mt[:], vt[:], TAU, xt_tiles[t + 1],
                    AluOpType.mult, AluOpType.add,
                )
                m_cur = mt[:]
    _split_multiwaits(nc)
    return nc


def kernel(x: np.ndarray) -> np.ndarray:
    global _cached_nc
    if _cached_nc is None:
        _cached_nc = _build()
    nc = _cached_nc

    x = np.ascontiguousarray(x, dtype=np.float32)
    assert x.shape == (T, B, N)
    xbt = np.ascontiguousarray(x.transpose(1, 0, 2))
    in_maps = [
        {"x": np.ascontiguousarray(xbt[:, :, k * NSH : (k + 1) * NSH])}
        for k in range(NCORES)
    ]
    res = run_bass_kernel_spmd(nc, in_maps, core_ids=list(range(NCORES)))
    global _last_exec_ns
    if res.exec_time_ns is not None:
        _last_exec_ns = res.exec_time_ns
    # per-core uint8 (B, T, NSH) -> f32 (T, B, N); spikes are exactly 0/1
    out = np.concatenate([r["y"] for r in res.results], axis=2)
    return np.ascontiguousarray(out.transpose(1, 0, 2)).astype(np.float32)


_last_exec_ns = None


# revision 9
# speedup vs baseline: 1.0231x; 1.0231x over previous
"""LIF forward recurrence on 8 Trainium2 NeuronCores — v10.

Input  x: (T=16, B=128, N=16384) float32, time-major.
    m[t] = tau * v[t-1] + x[t]      tau = 0.5
    y[t] = (m[t] >= 1)              spike
    v[t] = m[t] * (1 - y[t])        hard reset

Sharding: N split 8 ways (2048 per core); per-neuron recurrence, no
cross-core traffic.  Host re-lays each shard as (B, T, NSH) so chunked
DMAs read long contiguous runs per SBUF partition row.

Engine layout (measured costs per [128x2048] f32 tile):
  DVE    v = (m < 1) * m            stt 2.29us   } the serial chain,
         m' = 0.5*v + x[t+1]        stt 2.29us   } back-to-back on DVE
  Act    a = Sign(1 - m)            2.0us  off-chain
         y = Sign(1 - a) -> uint8   2.0us  exact {0,1}, incl. m == 1
  Sync   y[t] streamed out per step (uint8, 0.25 MB/step)
The DVE pair is the hard floor: fp32 two-tensor ops have no fast DVE
mode, Act can't combine two tensors, GpSimd elementwise ops poison DVE
throughput when co-running (measured ~0.66 vs 0.96 col/ns), and PE can
only accumulate into PSUM (no DMA in/out of PSUM).  Total DMA
(16.8 MB in + 4.2 MB out = ~60us) hides under the 71us DVE stream;
everything else is ramp/tail, minimized by issuing the first input
chunk from the Act queue right after the init barrier and streaming
spikes out every step (no end-of-kernel drain).

Output is bit-exact vs the f32 reference (the double-Sign gives exact
0/1 including m == 1.0, and the v-chain rounds identically).
"""

import numpy as np

import concourse.bass as bass
import concourse.mybir as mybir
from concourse.bass_utils import run_bass_kernel_spmd
from concourse.mybir import AluOpType
from concourse.tile import TileContext

T, B, N = 16, 128, 16384
NCORES = 8
NSH = N // NCORES  # 2048 neurons per core
TAU = 0.5
V_TH = 1.0

IN_CHUNKS = [1, 1, 2, 4, 4, 2, 1, 1]

_cached_nc = None


def _split_multiwaits(nc):
    """Walrus codegen supports only ONE sync-wait per instruction; Tile
    sometimes attaches more.  Move extras onto same-engine NoOps."""
    multi_ok = (mybir.InstEventSemaphore, mybir.InstNoOp)
    for f in nc.m.functions:
        for b in f.blocks:
            new_insts = []
            for inst in b.instructions:
                si = inst.sync_info
                if (
                    not isinstance(inst, multi_ok)
                    and si is not None
                    and len(si.on_wait) > 1
                ):
                    waits = list(si.on_wait)
                    for j, w in enumerate(waits[:-1]):
                        new_insts.append(
                            mybir.InstNoOp(
                                name=f"{inst.name}_presync{j}",
                                engine=inst.engine,
                                sync_info=mybir.SyncInfo(on_wait=[w], on_update=[]),
                            )
                        )
                    inst.sync_info = mybir.SyncInfo(
                        on_wait=[waits[-1]], on_update=list(si.on_update)
                    )
                new_insts.append(inst)
            b.instructions = new_insts


def _build():
    nc = bass.Bass(trn_type="TRN2")
    x = nc.dram_tensor("x", [B, T, NSH], mybir.dt.float32, kind="ExternalInput")
    y = nc.dram_tensor("y", [B, T, NSH], mybir.dt.uint8, kind="ExternalOutput")

    with TileContext(nc) as tc:
        with (
            tc.tile_pool(name="xin", bufs=2) as xin_pool,
            tc.tile_pool(name="mst", bufs=1) as m_pool,
            tc.tile_pool(name="vst", bufs=1) as v_pool,
            tc.tile_pool(name="sst", bufs=1) as s_pool,
        ):
            # chunk tiles; first chunk issued from the Act queue (starts
            # right after the init barrier, ~4us before GpSimd's DGE
            # preamble finishes), the rest from GpSimd's SWDGE path.
            xt_tiles = {}
            xts = []
            t0 = 0
            for ci, w in enumerate(IN_CHUNKS):
                xt = xin_pool.tile(
                    [B, 4, NSH], mybir.dt.float32, tag="xt", name=f"xt{ci}"
                )
                xts.append((xt, t0, w))
                for k in range(w):
                    xt_tiles[t0 + k] = xt[:, k, :]
                t0 += w
            for xt, a0, w in xts[:3]:
                nc.scalar.dma_start(out=xt[:, :w, :], in_=x[:, a0 : a0 + w, :])
            for xt, a0, w in xts[3:]:
                nc.gpsimd.dma_start(out=xt[:, :w, :], in_=x[:, a0 : a0 + w, :])

            m_cur = xt_tiles[0]
            for t in range(T):
                if t == T - 1:
                    yt = s_pool.tile(
                        [B, NSH], mybir.dt.uint8, tag="y", bufs=3, name="y_f"
                    )
                    nc.vector.tensor_scalar(
                        yt[:], m_cur, float(V_TH), None, AluOpType.is_ge
                    )
                    nc.sync.dma_start(out=y[:, t, :], in_=yt[:])
                    continue
                # spikes, exact 0/1 via double Sign (correct at m == 1)
                at = s_pool.tile(
                    [B, NSH], mybir.dt.float32, tag="a", bufs=2, name=f"a_{t}"
                )
                nc.scalar.activation(
                    at[:], m_cur, mybir.ActivationFunctionType.Sign,
                    bias=V_TH, scale=-1.0,
                )
                yt = s_pool.tile(
                    [B, NSH], mybir.dt.uint8, tag="y", bufs=3, name=f"y_{t}"
                )
                nc.scalar.activation(
                    yt[:], at[:], mybir.ActivationFunctionType.Sign,
                    bias=1.0, scale=-1.0,
                )
                nc.sync.dma_start(out=y[:, t, :], in_=yt[:])
                # the serial chain, back-to-back on DVE
                vt = v_pool.tile(
                    [B, NSH], mybir.dt.float32, tag="v", bufs=3, name=f"v_{t}"
                )
                nc.vector.scalar_tensor_tensor(
                    vt[:], m_cur, V_TH, m_cur, AluOpType.is_lt, AluOpType.mult
                )
                mt = m_pool.tile(
                    [B, NSH], mybir.dt.float32, tag="m", bufs=3,
                    name=f"m_{t + 1}",
                )
                nc.vector.scalar_tensor_tensor(
                    mt[:], vt[:], TAU, xt_tiles[t + 1],
                    AluOpType.mult, AluOpType.add,
                )
                m_cur = mt[:]
    _split_multiwaits(nc)
    return nc


def kernel(x: np.ndarray) -> np.ndarray:
    global _cached_nc
    if _cached_nc is None:
        _cached_nc = _build()
    nc = _cached_nc

    x = np.ascontiguousarray(x, dtype=np.float32)
    assert x.shape == (T, B, N)
    xbt = np.ascontiguousarray(x.transpose(1, 0, 2))
    in_maps = [
        {"x": np.ascontiguousarray(xbt[:, :, k * NSH : (k + 1) * NSH])}
        for k in range(NCORES)
    ]
    res = run_bass_kernel_spmd(nc, in_maps, core_ids=list(range(NCORES)))
    global _last_exec_ns
    if res.exec_time_ns is not None:
        _last_exec_ns = res.exec_time_ns
    # per-core uint8 (B, T, NSH) -> f32 (T, B, N); spikes are exactly 0/1
    out = np.concatenate([r["y"] for r in res.results], axis=2)
    return np.ascontiguousarray(out.transpose(1, 0, 2)).astype(np.float32)


_last_exec_ns = None


# revision 11
# speedup vs baseline: 1.0582x; 1.0344x over previous
"""LIF forward recurrence on 8 Trainium2 NeuronCores — v10.

Input  x: (T=16, B=128, N=16384) float32, time-major.
    m[t] = tau * v[t-1] + x[t]      tau = 0.5
    y[t] = (m[t] >= 1)              spike
    v[t] = m[t] * (1 - y[t])        hard reset

Sharding: N split 8 ways (2048 per core); per-neuron recurrence, no
cross-core traffic.  Host re-lays each shard as (B, T, NSH) so chunked
DMAs read long contiguous runs per SBUF partition row.

Engine layout (measured costs per [128x2048] f32 tile):
  DVE    v = (m < 1) * m            stt 2.29us   } the serial chain,
         m' = 0.5*v + x[t+1]        stt 2.29us   } back-to-back on DVE
  Act    a = Sign(1 - m)            2.0us  off-chain
         y = Sign(1 - a) -> uint8   2.0us  exact {0,1}, incl. m == 1
  Sync   y[t] streamed out per step (uint8, 0.25 MB/step)
The DVE pair is the hard floor: fp32 two-tensor ops have no fast DVE
mode, Act can't combine two tensors, GpSimd elementwise ops poison DVE
throughput when co-running (measured ~0.66 vs 0.96 col/ns), and PE can
only accumulate into PSUM (no DMA in/out of PSUM).  Total DMA
(16.8 MB in + 4.2 MB out = ~60us) hides under the 71us DVE stream;
everything else is ramp/tail, minimized by issuing the first input
chunk from the Act queue right after the init barrier and streaming
spikes out every step (no end-of-kernel drain).

Output is bit-exact vs the f32 reference (the double-Sign gives exact
0/1 including m == 1.0, and the v-chain rounds identically).
"""

import numpy as np

import concourse.bass as bass
import concourse.mybir as mybir
from concourse.bass_utils import run_bass_kernel_spmd
from concourse.mybir import AluOpType
from concourse.tile import TileContext

T, B, N = 16, 128, 16384
NCORES = 8
NSH = N // NCORES  # 2048 neurons per core
TAU = 0.5
V_TH = 1.0

IN_CHUNKS = [1, 1, 2, 4, 4, 2, 1, 1]

_cached_nc = None


def _split_multiwaits(nc):
    """Walrus codegen supports only ONE sync-wait per instruction; Tile
    sometimes attaches more.  Move extras onto same-engine NoOps."""
    multi_ok = (mybir.InstEventSemaphore, mybir.InstNoOp)
    for f in nc.m.functions:
        for b in f.blocks:
            new_insts = []
            for inst in b.instructions:
                si = inst.sync_info
                if (
                    not isinstance(inst, multi_ok)
                    and si is not None
                    and len(si.on_wait) > 1
                ):
                    waits = list(si.on_wait)
                    for j, w in enumerate(waits[:-1]):
                        new_insts.append(
                            mybir.InstNoOp(
                                name=f"{inst.name}_presync{j}",
                                engine=inst.engine,
                                sync_info=mybir.SyncInfo(on_wait=[w], on_update=[]),
                            )
                        )
                    inst.sync_info = mybir.SyncInfo(
                        on_wait=[waits[-1]], on_update=list(si.on_update)
                    )
                new_insts.append(inst)
            b.instructions = new_insts


def _build():
    nc = bass.Bass(trn_type="TRN2")
    x = nc.dram_tensor("x", [B, T, NSH], mybir.dt.float32, kind="ExternalInput")
    y = nc.dram_tensor("y", [B, T, NSH], mybir.dt.uint8, kind="ExternalOutput")

    with TileContext(nc) as tc:
        with (
            tc.tile_pool(name="xin", bufs=2) as xin_pool,
            tc.tile_pool(name="mst", bufs=1) as m_pool,
            tc.tile_pool(name="vst", bufs=1) as v_pool,
            tc.tile_pool(name="sst", bufs=1) as s_pool,
        ):
            # chunk tiles; first chunk issued from the Act queue (starts
            # right after the init barrier, ~4us before GpSimd's DGE
            # preamble finishes), the rest from GpSimd's SWDGE path.
            xt_tiles = {}
            xts = []
            t0 = 0
            for ci, w in enumerate(IN_CHUNKS):
                xt = xin_pool.tile(
                    [B, 4, NSH], mybir.dt.float32, tag="xt", name=f"xt{ci}"
                )
                xts.append((xt, t0, w))
                for k in range(w):
                    xt_tiles[t0 + k] = xt[:, k, :]
                t0 += w
            # chunks 0-2 on three separate HWDGE queues (DVE starts its
            # preamble earliest; one queue alone can't feed the early
            # steps), the rest on GpSimd's SWDGE path.
            for (xt, a0, w), eng in zip(xts[:3], [nc.sync, nc.scalar, nc.scalar]):
                eng.dma_start(out=xt[:, :w, :], in_=x[:, a0 : a0 + w, :])
            for xt, a0, w in xts[3:]:
                nc.gpsimd.dma_start(out=xt[:, :w, :], in_=x[:, a0 : a0 + w, :])

            m_cur = xt_tiles[0]
            for t in range(T):
                if t == T - 1:
                    yt = s_pool.tile(
                        [B, NSH], mybir.dt.uint8, tag="y", bufs=3, name="y_f"
                    )
                    nc.vector.tensor_scalar(
                        yt[:], m_cur, float(V_TH), None, AluOpType.is_ge
                    )
                    nc.sync.dma_start(out=y[:, t, :], in_=yt[:])
                    continue
                # spikes, exact 0/1 via double Sign (correct at m == 1)
                at = s_pool.tile(
                    [B, NSH], mybir.dt.float32, tag="a", bufs=2, name=f"a_{t}"
                )
                nc.scalar.activation(
                    at[:], m_cur, mybir.ActivationFunctionType.Sign,
                    bias=V_TH, scale=-1.0,
                )
                yt = s_pool.tile(
                    [B, NSH], mybir.dt.uint8, tag="y", bufs=3, name=f"y_{t}"
                )
                nc.scalar.activation(
                    yt[:], at[:], mybir.ActivationFunctionType.Sign,
                    bias=1.0, scale=-1.0,
                )
                nc.sync.dma_start(out=y[:, t, :], in_=yt[:])
                # the serial chain, back-to-back on DVE
                vt = v_pool.tile(
                    [B, NSH], mybir.dt.float32, tag="v", bufs=3, name=f"v_{t}"
                )
                nc.vector.scalar_tensor_tensor(
                    vt[:], m_cur, V_TH, m_cur, AluOpType.is_lt, AluOpType.mult
                )
                mt = m_pool.tile(
                    [B, NSH], mybir.dt.float32, tag="m", bufs=3,
                    name=f"m_{t + 1}",
                )
                nc.vector.scalar_tensor_tensor(
                    mt[:], vt[:], TAU, xt_tiles[t + 1],
                    AluOpType.mult, AluOpType.add,
                )
                m_cur = mt[:]
    _split_multiwaits(nc)
    return nc


def kernel(x: np.ndarray) -> np.ndarray:
    global _cached_nc
    if _cached_nc is None:
        _cached_nc = _build()
    nc = _cached_nc

    x = np.ascontiguousarray(x, dtype=np.float32)
    assert x.shape == (T, B, N)
    xbt = np.ascontiguousarray(x.transpose(1, 0, 2))
    in_maps = [
        {"x": np.ascontiguousarray(xbt[:, :, k * NSH : (k + 1) * NSH])}
        for k in range(NCORES)
    ]
    res = run_bass_kernel_spmd(nc, in_maps, core_ids=list(range(NCORES)))
    global _last_exec_ns
    if res.exec_time_ns is not None:
        _last_exec_ns = res.exec_time_ns
    # per-core uint8 (B, T, NSH) -> f32 (T, B, N); spikes are exactly 0/1
    out = np.concatenate([r["y"] for r in res.results], axis=2)
    return np.ascontiguousarray(out.transpose(1, 0, 2)).astype(np.float32)


_last_exec_ns = None
